# revision 14
# baseline (speedup 1.0000x reference)
"""AdMSoftmaxLoss fused distributed kernel for 8 TRN2 NeuronCores.

Math (reference):
    xn = x / ||x||                     # row-L2-normalized embeddings
    wf = xn @ W.T                      # [N, C] logits
    tgt = wf[i, y_i]
    num = S * (tgt - M)
    excl = sum_c exp(S*wf) - exp(S*tgt)
    L = num - log(exp(num) + excl);  loss = -mean(L)

Strategy: pure data-parallel over N (16384 rows -> 2048/core), no
collectives.  Each core computes its full [2048, 10000] logit block in
PSUM (bf16 matmul, fp32 accumulate) and applies exp with the ScalarEngine
activation, folding S/||x_i|| in as the per-partition activation scale.
Row sums of exp are split between the ACT accumulator (accum_out) and
VectorEngine reductions of the bf16 exp output, so ACT and DVE share the
reduction load.  The target logit is computed from a host-side gather
G = W[labels] via DVE dot products; per-row L values are DMA'd out and
the host just concatenates and means.

All device inputs are pre-shuffled on the host to partition-major
layouts so every DMA is a large contiguous transfer.
"""

import numpy as np
import ml_dtypes

import concourse.mybir as mybir
import concourse.tile as tile
from concourse import bacc
from concourse.bass_utils import run_bass_kernel_spmd

N, D, C = 16384, 256, 10000
S, M = 30.0, 0.4
NCORES = 8
NS = N // NCORES      # 2048 rows per core
NT = NS // 128        # 16 n-tiles of 128 rows
KT = D // 128         # 2 k-slices
CG = [2048, 2048, 2048, 2048, 1808]   # class-dim groups (sum = C)
assert sum(CG) == C

_F32 = mybir.dt.float32
_BF16 = mybir.dt.bfloat16
_I32 = mybir.dt.int32

N_WARMUP_MM = 6      # dummy matmuls to pull the PE HAM clock-gate to 8/8
ACT_ACCUM_EVERY = 4   # 1 of every this many groups reduced via ACT accum_out


def _build_nc(ns=NS, cg=tuple(CG), c=C):
    nt = ns // 128
    cg = list(cg)
    assert sum(cg) == c
    nc = bacc.Bacc("TRN2", target_bir_lowering=False)
    AF = mybir.ActivationFunctionType
    NT, C = nt, c  # noqa: N806 (shadow module constants for the body below)
    CG, NS = cg, ns  # noqa: N806
    NG = len(CG)  # noqa: N806
    NH = NT // 2  # noqa: N806
    mult = mybir.AluOpType.mult
    sub = mybir.AluOpType.subtract
    asr = mybir.AluOpType.arith_shift_right
    addop = mybir.AluOpType.add

    xt_ext = nc.declare_dram_parameter("xt", [128, KT, NS], _BF16, isOutput=False)
    wt_ext = nc.declare_dram_parameter("wt", [128, KT, C], _BF16, isOutput=False)
    xf_ext = nc.declare_dram_parameter("xf", [128, NT, D], _BF16, isOutput=False)
    g_ext = nc.declare_dram_parameter("g", [128, NT, D], _BF16, isOutput=False)
    out_ext = nc.declare_dram_parameter("out", [128, NT], _F32, isOutput=True)

    with tile.TileContext(nc) as tc:
        with (
            tc.tile_pool(name="big", bufs=1) as big,
            tc.tile_pool(name="stat", bufs=1) as stat,
            tc.tile_pool(name="scr", bufs=1) as scr,
            tc.tile_pool(name="expb", bufs=4) as expb,
            tc.tile_pool(name="psum", bufs=2, space="PSUM") as psum,
        ):
            # ---- prologue: warm the exp ACT table + the PE HAM clock-gate
            # while the first DMAs land ----
            wu_a = scr.tile([128, 128], _BF16)
            wu_b = scr.tile([128, 512], _BF16)
            wu_e = scr.tile([128, 1], _F32)
            nc.vector.memset(wu_a, 0.0)
            nc.vector.memset(wu_b, 0.0)
            nc.vector.memset(wu_e, 0.0)
            nc.scalar.activation(wu_e, wu_e, AF.Exp)  # pull exp table load early
            wu_p = psum.tile([128, 2048], _F32, tag="pt")
            for i in range(N_WARMUP_MM):
                nc.tensor.matmul(
                    wu_p[:, (i % 4) * 512 : (i % 4) * 512 + 512],
                    wu_a,
                    wu_b,
                    start=True,
                    stop=True,
                )

            # ---- input DMAs, ordered by when they gate compute ----
            xf_sb = big.tile([128, NT, D], _BF16)
            g_sb = big.tile([128, NT, D], _BF16)
            wt_sb = big.tile([128, KT, C], _BF16)
            xt_sb = big.tile([128, KT, NS], _BF16)

            def _wt_chunk(gi):
                c0 = sum(CG[:gi])
                w = CG[gi]
                for k in range(KT):
                    nc.sync.dma_start(
                        out=wt_sb[:, k, c0 : c0 + w], in_=wt_ext[:, k, c0 : c0 + w]
                    )

            # critical chain: xf half 0 -> xt t0 cols -> wt chunk 0
            nc.sync.dma_start(out=xf_sb[:, :NH, :], in_=xf_ext[:, :NH, :])
            for k in range(KT):
                nc.sync.dma_start(out=xt_sb[:, k, :128], in_=xt_ext[:, k, :128])
            _wt_chunk(0)
            nc.sync.dma_start(out=xf_sb[:, NH:, :], in_=xf_ext[:, NH:, :])
            for k in range(KT):
                nc.sync.dma_start(out=xt_sb[:, k, 128:], in_=xt_ext[:, k, 128:])
            nc.sync.dma_start(out=g_sb[:, :NH, :], in_=g_ext[:, :NH, :])
            nc.sync.dma_start(out=g_sb[:, NH:, :], in_=g_ext[:, NH:, :])
            for gi in range(1, NG):
                _wt_chunk(gi)

            # ---- phase 1: ||x||^2 per row, then S/||x|| via a DVE-only
            # Newton rsqrt (no ACT table switch), in halves so the exp
            # stream can start after the first half ----
            ss = stat.tile([128, NT], _F32)
            sr = stat.tile([128, NT], _F32)      # S / ||x||
            sq_scr = scr.tile([128, NH, D], _F32)
            yv = stat.tile([128, NT], _F32)
            t1 = stat.tile([128, NT], _F32)
            t2 = stat.tile([128, NT], _F32)

            def _rsqrt(lo, hi, out_scale):
                ssh = ss[:, lo:hi]
                yh, t1h, t2h = yv[:, lo:hi], t1[:, lo:hi], t2[:, lo:hi]
                # quake seed: y0 = bitcast(0x5f3759df - (bitcast(ss) >> 1))
                nc.vector.tensor_scalar(
                    t1h.bitcast(_I32), ssh.bitcast(_I32), 1, None, asr
                )
                nc.vector.tensor_scalar(
                    yh.bitcast(_I32), t1h.bitcast(_I32), 0x5F3759DF, -1, sub, mult
                )
                for _ in range(2):  # Newton: y *= 1.5 - 0.5*ss*y^2
                    nc.vector.tensor_mul(t1h, yh, yh)
                    nc.vector.tensor_mul(t2h, t1h, ssh)
                    nc.vector.tensor_scalar(t1h, t2h, -0.5, 1.5, mult, addop)
                    nc.vector.tensor_mul(yh, yh, t1h)
                nc.vector.tensor_scalar_mul(out_scale[:, lo:hi], yh, S)

            def _ph1_half(h):
                lo, hi = h * NH, (h + 1) * NH
                nc.vector.tensor_mul(sq_scr, xf_sb[:, lo:hi, :], xf_sb[:, lo:hi, :])
                nc.vector.reduce_sum(
                    ss[:, lo:hi], sq_scr, axis=mybir.AxisListType.X
                )
                _rsqrt(lo, hi, sr)

            # ---- phase 2: logits + exp; row-sums split ACT-accum / DVE ----
            esum = stat.tile([128, NT * NG], _F32)

            def _exp_iter(gi, w, t, force_accum=False):
                c0 = sum(CG[:gi])
                it = gi * NT + t
                pt = psum.tile([128, 2048], _F32, tag="pt")
                for b0 in range(0, w, 512):
                    bw = min(512, w - b0)
                    for k in range(KT):
                        nc.tensor.matmul(
                            pt[:, b0 : b0 + bw],
                            xt_sb[:, k, t * 128 : (t + 1) * 128],
                            wt_sb[:, k, c0 + b0 : c0 + b0 + bw],
                            start=(k == 0),
                            stop=(k == KT - 1),
                        )
                idx = t * NG + gi
                eo = expb.tile([128, 2048], _BF16, tag="ex")
                if force_accum or it % ACT_ACCUM_EVERY == 0:
                    nc.scalar.activation(
                        eo[:, :w],
                        pt[:, :w],
                        AF.Exp,
                        scale=sr[:, t : t + 1],
                        accum_out=esum[:, idx : idx + 1],
                    )
                else:
                    nc.scalar.activation(
                        eo[:, :w], pt[:, :w], AF.Exp, scale=sr[:, t : t + 1]
                    )
                    nc.vector.reduce_sum(
                        esum[:, idx : idx + 1],
                        eo[:, :w],
                        axis=mybir.AxisListType.X,
                    )

            # program order = schedule order for Tile's semaphore thresholds:
            # keep the DVE work needed by later ACTs AHEAD of those ACTs, and
            # give the first half-pass ACT-accum reductions so the DVE queue
            # stays clear for phase-1 half 1 and the target-dot chain.
            _ph1_half(0)
            for t in range(NT // 2):
                _exp_iter(0, CG[0], t, force_accum=True)

            # Demote the remaining stats work in the scheduler's priority
            # order: its consumers are several ACT iterations away, and
            # letting it schedule early inflates the semaphore thresholds
            # the first exp activations wait on.
            with tc.high_priority(offset=-(10**6)):
                _ph1_half(1)
                rawt = stat.tile([128, NT], _F32)
                for h in range(2):
                    lo, hi = h * NH, (h + 1) * NH
                    nc.vector.tensor_mul(
                        sq_scr, xf_sb[:, lo:hi, :], g_sb[:, lo:hi, :]
                    )
                    nc.vector.reduce_sum(
                        rawt[:, lo:hi], sq_scr, axis=mybir.AxisListType.X
                    )
                st = stat.tile([128, NT], _F32)
                nc.vector.tensor_mul(st, sr, rawt)             # S * tgt
                num = stat.tile([128, NT], _F32)
                nc.vector.tensor_scalar_add(num, st, -S * M)   # S * (tgt - M)

            for t in range(NT // 2, NT):
                _exp_iter(0, CG[0], t)
            rest = [(gi, CG[gi]) for gi in range(1, NG)]
            if rest:
                gi1, w1 = rest[0]
                for t in range(NT):
                    _exp_iter(gi1, w1, t)

            expn = stat.tile([128, NT], _F32)
            nc.scalar.activation(expn, num, AF.Exp)
            expt = stat.tile([128, NT], _F32)
            nc.scalar.activation(expt, st, AF.Exp)

            for gi, w in rest[1:]:
                for t in range(NT):
                    _exp_iter(gi, w, t)

            # ---- phase 3: combine, log, write out ----
            esum_v = esum.rearrange("p (t g) -> p t g", g=NG)
            et = stat.tile([128, NT], _F32)
            nc.vector.reduce_sum(et, esum_v, axis=mybir.AxisListType.X)
            denom = stat.tile([128, NT], _F32)
            nc.vector.tensor_add(denom, et, expn)
            nc.vector.tensor_sub(denom, denom, expt)
            lg = stat.tile([128, NT], _F32)
            nc.scalar.activation(lg, denom, AF.Ln)
            L = stat.tile([128, NT], _F32)
            nc.vector.tensor_sub(L, num, lg)
            nc.sync.dma_start(out=out_ext[:], in_=L)

    nc.finalize()
    return nc


_NC_CACHE = None


def _get_nc():
    global _NC_CACHE
    if _NC_CACHE is None:
        _NC_CACHE = _build_nc()
    return _NC_CACHE


def _shuffle_pm(a, nt):
    """[nt*128, d] row-major -> [128, nt, d] partition-major."""
    d = a.shape[-1]
    return np.ascontiguousarray(a.reshape(nt, 128, d).transpose(1, 0, 2))


def prep_core(xs, ls, W, wt=None):
    """Build one core's input map from its row block. Layouts partition-major."""
    nt = xs.shape[0] // 128
    c = W.shape[0]
    if wt is None:
        wt = _shuffle_pm(np.ascontiguousarray(W.T), KT).astype(ml_dtypes.bfloat16)
    xt = _shuffle_pm(np.ascontiguousarray(xs.T), KT).astype(ml_dtypes.bfloat16)
    xf = _shuffle_pm(xs, nt).astype(ml_dtypes.bfloat16)
    g = _shuffle_pm(W[ls], nt).astype(ml_dtypes.bfloat16)
    return {"xt": xt, "wt": wt, "xf": xf, "g": g}


def make_in_maps(x, labels, W):
    x = np.asarray(x, dtype=np.float32)
    W = np.asarray(W, dtype=np.float32)
    labels = np.asarray(labels)
    wt = _shuffle_pm(np.ascontiguousarray(W.T), KT).astype(ml_dtypes.bfloat16)
    return [
        prep_core(
            x[i * NS : (i + 1) * NS], labels[i * NS : (i + 1) * NS], W, wt
        )
        for i in range(NCORES)
    ]


def run_device(x, labels, W, **kwargs):
    nc = _get_nc()
    in_maps = make_in_maps(x, labels, W)
    res = run_bass_kernel_spmd(nc, in_maps, list(range(NCORES)), **kwargs)
    return res


def finish(res):
    parts = []
    for i in range(NCORES):
        o = res.results[i]["out"]            # [128, NT]; row = t*128 + p
        parts.append(np.asarray(o).T.reshape(-1))
    L = np.concatenate(parts)
    return np.asarray(-np.mean(L), dtype=np.float32)


def kernel(x, labels, W):
    return finish(run_device(x, labels, W))


# revision 19
# speedup vs baseline: 1.0353x; 1.0353x over previous
"""AdMSoftmaxLoss fused distributed kernel for 8 TRN2 NeuronCores.

Math (reference):
    xn = x / ||x||                     # row-L2-normalized embeddings
    wf = xn @ W.T                      # [N, C] logits
    tgt = wf[i, y_i]
    num = S * (tgt - M)
    excl = sum_c exp(S*wf) - exp(S*tgt)
    L = num - log(exp(num) + excl);  loss = -mean(L)

Strategy: pure data-parallel over N (16384 rows -> 2048/core), no
collectives.  Each core computes its full [2048, 10000] logit block in
PSUM (bf16 matmul, fp32 accumulate) and applies exp with the ScalarEngine
activation, folding S/||x_i|| in as the per-partition activation scale.
Row sums of exp are split between the ACT accumulator (accum_out) and
VectorEngine reductions of the bf16 exp output, so ACT and DVE share the
reduction load.  The target logit is computed from a host-side gather
G = W[labels] via DVE dot products; per-row L values are DMA'd out and
the host just concatenates and means.

All device inputs are pre-shuffled on the host to partition-major
layouts so every DMA is a large contiguous transfer.
"""

import numpy as np
import ml_dtypes

import concourse.mybir as mybir
import concourse.tile as tile
from concourse import bacc
from concourse.bass_utils import run_bass_kernel_spmd

N, D, C = 16384, 256, 10000
S, M = 30.0, 0.4
NCORES = 8
NS = N // NCORES      # 2048 rows per core
NT = NS // 128        # 16 n-tiles of 128 rows
KT = D // 128         # 2 k-slices
CG = [2048, 2048, 2048, 2048, 1808]   # class-dim groups (sum = C)
assert sum(CG) == C

_F32 = mybir.dt.float32
_BF16 = mybir.dt.bfloat16
_I32 = mybir.dt.int32

N_WARMUP_MM = 6      # dummy matmuls to pull the PE HAM clock-gate to 8/8
ACT_ACCUM_EVERY = 4   # 1 of every this many groups reduced via ACT accum_out


def _build_nc(ns=NS, cg=tuple(CG), c=C):
    nt = ns // 128
    cg = list(cg)
    assert sum(cg) == c
    nc = bacc.Bacc("TRN2", target_bir_lowering=False)
    AF = mybir.ActivationFunctionType
    NT, C = nt, c  # noqa: N806 (shadow module constants for the body below)
    CG, NS = cg, ns  # noqa: N806
    NG = len(CG)  # noqa: N806
    NH = NT // 2  # noqa: N806
    mult = mybir.AluOpType.mult
    sub = mybir.AluOpType.subtract
    asr = mybir.AluOpType.arith_shift_right
    addop = mybir.AluOpType.add

    xt_ext = nc.declare_dram_parameter("xt", [128, KT, NS], _BF16, isOutput=False)
    wt_ext = nc.declare_dram_parameter("wt", [128, KT, C], _BF16, isOutput=False)
    xf_ext = nc.declare_dram_parameter("xf", [128, NT, D], _BF16, isOutput=False)
    g_ext = nc.declare_dram_parameter("g", [128, NT, D], _BF16, isOutput=False)
    out_ext = nc.declare_dram_parameter("out", [128, NT], _F32, isOutput=True)

    with tile.TileContext(nc) as tc:
        with (
            tc.tile_pool(name="big", bufs=1) as big,
            tc.tile_pool(name="stat", bufs=1) as stat,
            tc.tile_pool(name="scr", bufs=1) as scr,
            tc.tile_pool(name="expb", bufs=4) as expb,
            tc.tile_pool(name="psum", bufs=2, space="PSUM") as psum,
        ):
            # ---- prologue: warm the exp ACT table + the PE HAM clock-gate
            # while the first DMAs land ----
            wu_a = scr.tile([128, 128], _BF16)
            wu_b = scr.tile([128, 512], _BF16)
            wu_e = scr.tile([128, 1], _F32)
            nc.vector.memset(wu_a, 0.0)
            nc.vector.memset(wu_b, 0.0)
            nc.vector.memset(wu_e, 0.0)
            nc.scalar.activation(wu_e, wu_e, AF.Exp)  # pull exp table load early
            wu_p = psum.tile([128, 2048], _F32, tag="pt")
            for i in range(N_WARMUP_MM):
                nc.tensor.matmul(
                    wu_p[:, (i % 4) * 512 : (i % 4) * 512 + 512],
                    wu_a,
                    wu_b,
                    start=True,
                    stop=True,
                )

            # ---- input DMAs, ordered by when they gate compute ----
            xf_sb = big.tile([128, NT, D], _BF16)
            g_sb = big.tile([128, NT, D], _BF16)
            wt_sb = big.tile([128, KT, C], _BF16)
            xt_sb = big.tile([128, KT, NS], _BF16)

            def _wt_chunk(gi):
                c0 = sum(CG[:gi])
                w = CG[gi]
                for k in range(KT):
                    nc.sync.dma_start(
                        out=wt_sb[:, k, c0 : c0 + w], in_=wt_ext[:, k, c0 : c0 + w]
                    )

            # critical chain: xf half 0 -> xt t0 cols -> wt chunk 0
            nc.sync.dma_start(out=xf_sb[:, :NH, :], in_=xf_ext[:, :NH, :])
            for k in range(KT):
                nc.sync.dma_start(out=xt_sb[:, k, :128], in_=xt_ext[:, k, :128])
            _wt_chunk(0)
            nc.sync.dma_start(out=xf_sb[:, NH:, :], in_=xf_ext[:, NH:, :])
            for k in range(KT):
                nc.sync.dma_start(out=xt_sb[:, k, 128:], in_=xt_ext[:, k, 128:])
            nc.sync.dma_start(out=g_sb[:, :NH, :], in_=g_ext[:, :NH, :])
            nc.sync.dma_start(out=g_sb[:, NH:, :], in_=g_ext[:, NH:, :])
            for gi in range(1, NG):
                _wt_chunk(gi)

            # ---- phase 1: ||x||^2 per row, then S/||x|| via a DVE-only
            # Newton rsqrt (no ACT table switch), in halves so the exp
            # stream can start after the first half ----
            ss = stat.tile([128, NT], _F32)
            sr = stat.tile([128, NT], _F32)      # S / ||x||
            sq_scr = scr.tile([128, NH, D], _F32)
            yv = stat.tile([128, NT], _F32)
            t1 = stat.tile([128, NT], _F32)
            t2 = stat.tile([128, NT], _F32)

            def _rsqrt(lo, hi, out_scale):
                ssh = ss[:, lo:hi]
                yh, t1h, t2h = yv[:, lo:hi], t1[:, lo:hi], t2[:, lo:hi]
                # quake seed: y0 = bitcast(0x5f3759df - (bitcast(ss) >> 1))
                nc.vector.tensor_scalar(
                    t1h.bitcast(_I32), ssh.bitcast(_I32), 1, None, asr
                )
                nc.vector.tensor_scalar(
                    yh.bitcast(_I32), t1h.bitcast(_I32), 0x5F3759DF, -1, sub, mult
                )
                for _ in range(2):  # Newton: y *= 1.5 - 0.5*ss*y^2
                    nc.vector.tensor_mul(t1h, yh, yh)
                    nc.vector.tensor_mul(t2h, t1h, ssh)
                    nc.vector.tensor_scalar(t1h, t2h, -0.5, 1.5, mult, addop)
                    nc.vector.tensor_mul(yh, yh, t1h)
                nc.vector.tensor_scalar_mul(out_scale[:, lo:hi], yh, S)

            def _ph1_half(h):
                lo, hi = h * NH, (h + 1) * NH
                nc.vector.tensor_mul(sq_scr, xf_sb[:, lo:hi, :], xf_sb[:, lo:hi, :])
                nc.vector.reduce_sum(
                    ss[:, lo:hi], sq_scr, axis=mybir.AxisListType.X
                )
                _rsqrt(lo, hi, sr)

            # ---- phase 2: logits + exp; row-sums split ACT-accum / DVE ----
            esum = stat.tile([128, NT * NG], _F32)

            def _exp_iter(gi, w, t, force_accum=False):
                c0 = sum(CG[:gi])
                it = gi * NT + t
                pt = psum.tile([128, 2048], _F32, tag="pt")
                for b0 in range(0, w, 512):
                    bw = min(512, w - b0)
                    for k in range(KT):
                        nc.tensor.matmul(
                            pt[:, b0 : b0 + bw],
                            xt_sb[:, k, t * 128 : (t + 1) * 128],
                            wt_sb[:, k, c0 + b0 : c0 + b0 + bw],
                            start=(k == 0),
                            stop=(k == KT - 1),
                        )
                idx = t * NG + gi
                eo = expb.tile([128, 2048], _BF16, tag="ex")
                if force_accum or it % ACT_ACCUM_EVERY == 0:
                    act = nc.scalar.activation(
                        eo[:, :w],
                        pt[:, :w],
                        AF.Exp,
                        scale=sr[:, t : t + 1],
                        accum_out=esum[:, idx : idx + 1],
                    )
                else:
                    act = nc.scalar.activation(
                        eo[:, :w], pt[:, :w], AF.Exp, scale=sr[:, t : t + 1]
                    )
                    nc.vector.reduce_sum(
                        esum[:, idx : idx + 1],
                        eo[:, :w],
                        axis=mybir.AxisListType.X,
                    )
                return act

            # program order = schedule order for Tile's semaphore thresholds:
            # keep the DVE work needed by later ACTs AHEAD of those ACTs, and
            # give the first half-pass ACT-accum reductions so the DVE queue
            # stays clear for phase-1 half 1 and the target-dot chain.
            _ph1_half(0)
            acts0 = [
                _exp_iter(0, CG[0], t, force_accum=True) for t in range(NT // 2)
            ]

            # The scheduler's cost model doesn't see real DMA latency, so it
            # would pack the remaining stats work ahead of the first exp
            # activations, inflating the semaphore thresholds they wait on.
            # Pin the chain heads behind early stream ACTs instead.
            h1_mul = nc.vector.tensor_mul(
                sq_scr, xf_sb[:, NH:, :], xf_sb[:, NH:, :]
            )
            tile.add_dep_helper(
                h1_mul.ins, acts0[min(1, len(acts0) - 1)].ins, sync=False,
                reason="phase1-h1 after early exp stream",
            )
            nc.vector.reduce_sum(ss[:, NH:], sq_scr, axis=mybir.AxisListType.X)
            _rsqrt(NH, NT, sr)

            rawt = stat.tile([128, NT], _F32)
            for h in range(2):
                lo, hi = h * NH, (h + 1) * NH
                rmul = nc.vector.tensor_mul(
                    sq_scr, xf_sb[:, lo:hi, :], g_sb[:, lo:hi, :]
                )
                tile.add_dep_helper(
                    rmul.ins, acts0[min(3 + h, len(acts0) - 1)].ins, sync=False,
                    reason="target-dot after early exp stream",
                )
                nc.vector.reduce_sum(
                    rawt[:, lo:hi], sq_scr, axis=mybir.AxisListType.X
                )
            st = stat.tile([128, NT], _F32)
            nc.vector.tensor_mul(st, sr, rawt)             # S * tgt
            num = stat.tile([128, NT], _F32)
            nc.vector.tensor_scalar_add(num, st, -S * M)   # S * (tgt - M)

            for t in range(NT // 2, NT):
                _exp_iter(0, CG[0], t)
            rest = [(gi, CG[gi]) for gi in range(1, NG)]
            if rest:
                gi1, w1 = rest[0]
                for t in range(NT):
                    _exp_iter(gi1, w1, t)

            expn = stat.tile([128, NT], _F32)
            nc.scalar.activation(expn, num, AF.Exp)
            expt = stat.tile([128, NT], _F32)
            nc.scalar.activation(expt, st, AF.Exp)

            for gi, w in rest[1:]:
                for t in range(NT):
                    _exp_iter(gi, w, t)

            # ---- phase 3: combine, log (DVE bit-trick — avoids an ACT
            # table reload on the tail), write out ----
            esum_v = esum.rearrange("p (t g) -> p t g", g=NG)
            et = stat.tile([128, NT], _F32)
            nc.vector.reduce_sum(et, esum_v, axis=mybir.AxisListType.X)
            denom = stat.tile([128, NT], _F32)
            nc.vector.tensor_add(denom, et, expn)
            nc.vector.tensor_sub(denom, denom, expt)
            # ln(d) = ln2 * (e + log2(m)), d = m * 2^e, m in [1,2)
            lsr = mybir.AluOpType.logical_shift_right
            band = mybir.AluOpType.bitwise_and
            bor = mybir.AluOpType.bitwise_or
            ef = stat.tile([128, NT], _F32)
            mm = stat.tile([128, NT], _F32)
            acc = stat.tile([128, NT], _F32)
            nc.vector.tensor_scalar(
                acc.bitcast(_I32), denom.bitcast(_I32), 23, None, lsr
            )
            nc.vector.tensor_scalar(
                acc.bitcast(_I32), acc.bitcast(_I32), 127, None, sub
            )
            nc.vector.tensor_copy(ef, acc.bitcast(_I32))      # int -> float
            nc.vector.tensor_scalar(
                mm.bitcast(_I32), denom.bitcast(_I32), 0x7FFFFF, 0x3F800000,
                band, bor,
            )
            # log2(m) via degree-4 poly (max abs err 3.4e-4)
            PC = [
                -7.6181190215e-02, 6.0924791153e-01, -2.0338020960e+00,
                3.9788399321e+00, -2.4777674281e+00,
            ]
            nc.vector.tensor_scalar(acc, mm, PC[0], PC[1], mult, addop)
            for ci in PC[2:]:
                nc.vector.tensor_mul(acc, acc, mm)
                nc.vector.tensor_scalar_add(acc, acc, ci)
            nc.vector.tensor_add(acc, acc, ef)                # e + log2(m)
            nc.vector.tensor_scalar_mul(acc, acc, float(np.log(2.0)))
            L = stat.tile([128, NT], _F32)
            nc.vector.tensor_sub(L, num, acc)
            nc.sync.dma_start(out=out_ext[:], in_=L)

    nc.finalize()
    return nc


_NC_CACHE = None


def _get_nc():
    global _NC_CACHE
    if _NC_CACHE is None:
        _NC_CACHE = _build_nc()
    return _NC_CACHE


def _shuffle_pm(a, nt):
    """[nt*128, d] row-major -> [128, nt, d] partition-major."""
    d = a.shape[-1]
    return np.ascontiguousarray(a.reshape(nt, 128, d).transpose(1, 0, 2))


def prep_core(xs, ls, W, wt=None):
    """Build one core's input map from its row block. Layouts partition-major."""
    nt = xs.shape[0] // 128
    c = W.shape[0]
    if wt is None:
        wt = _shuffle_pm(np.ascontiguousarray(W.T), KT).astype(ml_dtypes.bfloat16)
    xt = _shuffle_pm(np.ascontiguousarray(xs.T), KT).astype(ml_dtypes.bfloat16)
    xf = _shuffle_pm(xs, nt).astype(ml_dtypes.bfloat16)
    g = _shuffle_pm(W[ls], nt).astype(ml_dtypes.bfloat16)
    return {"xt": xt, "wt": wt, "xf": xf, "g": g}


def make_in_maps(x, labels, W):
    x = np.asarray(x, dtype=np.float32)
    W = np.asarray(W, dtype=np.float32)
    labels = np.asarray(labels)
    wt = _shuffle_pm(np.ascontiguousarray(W.T), KT).astype(ml_dtypes.bfloat16)
    return [
        prep_core(
            x[i * NS : (i + 1) * NS], labels[i * NS : (i + 1) * NS], W, wt
        )
        for i in range(NCORES)
    ]


def run_device(x, labels, W, **kwargs):
    nc = _get_nc()
    in_maps = make_in_maps(x, labels, W)
    res = run_bass_kernel_spmd(nc, in_maps, list(range(NCORES)), **kwargs)
    return res


def finish(res):
    parts = []
    for i in range(NCORES):
        o = res.results[i]["out"]            # [128, NT]; row = t*128 + p
        parts.append(np.asarray(o).T.reshape(-1))
    L = np.concatenate(parts)
    return np.asarray(-np.mean(L), dtype=np.float32)


def kernel(x, labels, W):
    return finish(run_device(x, labels, W))


# revision 25
# speedup vs baseline: 1.0458x; 1.0102x over previous
"""AdMSoftmaxLoss fused distributed kernel for 8 TRN2 NeuronCores.

Math (reference):
    xn = x / ||x||                     # row-L2-normalized embeddings
    wf = xn @ W.T                      # [N, C] logits
    tgt = wf[i, y_i]
    num = S * (tgt - M)
    excl = sum_c exp(S*wf) - exp(S*tgt)
    L = num - log(exp(num) + excl);  loss = -mean(L)

Strategy: pure data-parallel over N (16384 rows -> 2048/core), no
collectives.  Each core computes its full [2048, 10000] logit block in
PSUM (bf16 matmul, fp32 accumulate) and applies exp with the ScalarEngine
activation, folding S/||x_i|| in as the per-partition activation scale.
Row sums of exp are split between the ACT accumulator (accum_out) and
VectorEngine reductions of the bf16 exp output, so ACT and DVE share the
reduction load.  The target logit is computed from a host-side gather
G = W[labels] via DVE dot products; per-row L values are DMA'd out and
the host just concatenates and means.

All device inputs are pre-shuffled on the host to partition-major
layouts so every DMA is a large contiguous transfer.
"""

import numpy as np
import ml_dtypes

import concourse.mybir as mybir
import concourse.tile as tile
from concourse import bacc
from concourse.bass_utils import run_bass_kernel_spmd

N, D, C = 16384, 256, 10000
S, M = 30.0, 0.4
NCORES = 8
NS = N // NCORES      # 2048 rows per core
NT = NS // 128        # 16 n-tiles of 128 rows
KT = D // 128         # 2 k-slices
CG = [2048, 2048, 2048, 2048, 1808]   # class-dim groups (sum = C)
assert sum(CG) == C

_F32 = mybir.dt.float32
_BF16 = mybir.dt.bfloat16
_I32 = mybir.dt.int32

N_WARMUP_MM = 6      # dummy matmuls to pull the PE HAM clock-gate to 8/8
ACT_ACCUM_EVERY = 6   # 1 of every this many groups reduced via ACT accum_out


def _build_nc(ns=NS, cg=tuple(CG), c=C):
    nt = ns // 128
    cg = list(cg)
    assert sum(cg) == c
    nc = bacc.Bacc("TRN2", target_bir_lowering=False)
    AF = mybir.ActivationFunctionType
    NT, C = nt, c  # noqa: N806 (shadow module constants for the body below)
    CG, NS = cg, ns  # noqa: N806
    NG = len(CG)  # noqa: N806
    NH = NT // 2  # noqa: N806
    mult = mybir.AluOpType.mult
    sub = mybir.AluOpType.subtract
    asr = mybir.AluOpType.arith_shift_right
    addop = mybir.AluOpType.add

    xt_ext = nc.declare_dram_parameter("xt", [128, KT, NS], _BF16, isOutput=False)
    wt_ext = nc.declare_dram_parameter("wt", [128, KT, C], _BF16, isOutput=False)
    xf_ext = nc.declare_dram_parameter("xf", [128, NT, D], _BF16, isOutput=False)
    g_ext = nc.declare_dram_parameter("g", [128, NT, D], _BF16, isOutput=False)
    out_ext = nc.declare_dram_parameter("out", [128, NT], _F32, isOutput=True)

    with tile.TileContext(nc) as tc:
        with (
            tc.tile_pool(name="big", bufs=1) as big,
            tc.tile_pool(name="stat", bufs=1) as stat,
            tc.tile_pool(name="scr", bufs=1) as scr,
            tc.tile_pool(name="expb", bufs=6) as expb,
            tc.tile_pool(name="psum", bufs=2, space="PSUM") as psum,
        ):
            # ---- prologue: warm the exp ACT table + the PE HAM clock-gate
            # while the first DMAs land ----
            wu_a = scr.tile([128, 128], _BF16)
            wu_b = scr.tile([128, 512], _BF16)
            wu_e = scr.tile([128, 1], _F32)
            nc.vector.memset(wu_a, 0.0)
            nc.vector.memset(wu_b, 0.0)
            nc.vector.memset(wu_e, 0.0)
            nc.scalar.activation(wu_e, wu_e, AF.Exp)  # pull exp table load early
            wu_p = psum.tile([128, 2048], _F32, tag="pt")
            for i in range(N_WARMUP_MM):
                nc.tensor.matmul(
                    wu_p[:, (i % 4) * 512 : (i % 4) * 512 + 512],
                    wu_a,
                    wu_b,
                    start=True,
                    stop=True,
                )

            # ---- input DMAs, ordered by when they gate compute ----
            xf_sb = big.tile([128, NT, D], _BF16)
            g_sb = big.tile([128, NT, D], _BF16)
            wt_sb = big.tile([128, KT, C], _BF16)
            xt_sb = big.tile([128, KT, NS], _BF16)

            def _wt_chunk(gi):
                c0 = sum(CG[:gi])
                w = CG[gi]
                for k in range(KT):
                    nc.sync.dma_start(
                        out=wt_sb[:, k, c0 : c0 + w], in_=wt_ext[:, k, c0 : c0 + w]
                    )

            # critical chain: xf quarter 0 -> xt t0 cols -> wt chunk 0
            NQ = max(NT // 4, 1)  # noqa: N806
            nc.sync.dma_start(out=xf_sb[:, :NQ, :], in_=xf_ext[:, :NQ, :])
            for k in range(KT):
                nc.sync.dma_start(out=xt_sb[:, k, :128], in_=xt_ext[:, k, :128])
            _wt_chunk(0)
            if NQ < NH:
                nc.sync.dma_start(out=xf_sb[:, NQ:NH, :], in_=xf_ext[:, NQ:NH, :])
            nc.sync.dma_start(out=xf_sb[:, NH:, :], in_=xf_ext[:, NH:, :])
            for k in range(KT):
                nc.sync.dma_start(out=xt_sb[:, k, 128:], in_=xt_ext[:, k, 128:])
            nc.sync.dma_start(out=g_sb[:, :NH, :], in_=g_ext[:, :NH, :])
            nc.sync.dma_start(out=g_sb[:, NH:, :], in_=g_ext[:, NH:, :])
            for gi in range(1, NG):
                _wt_chunk(gi)

            # ---- phase 1: ||x||^2 per row, then S/||x|| via a DVE-only
            # Newton rsqrt (no ACT table switch), in halves so the exp
            # stream can start after the first half ----
            ss = stat.tile([128, NT], _F32)
            sr = stat.tile([128, NT], _F32)      # S / ||x||
            sq_scr = scr.tile([128, NH, D], _F32)
            yv = stat.tile([128, NT], _F32)
            t1 = stat.tile([128, NT], _F32)
            t2 = stat.tile([128, NT], _F32)

            def _rsqrt(lo, hi, out_scale):
                ssh = ss[:, lo:hi]
                yh, t1h, t2h = yv[:, lo:hi], t1[:, lo:hi], t2[:, lo:hi]
                # quake seed: y0 = bitcast(0x5f3759df - (bitcast(ss) >> 1))
                nc.vector.tensor_scalar(
                    t1h.bitcast(_I32), ssh.bitcast(_I32), 1, None, asr
                )
                nc.vector.tensor_scalar(
                    yh.bitcast(_I32), t1h.bitcast(_I32), 0x5F3759DF, -1, sub, mult
                )
                for _ in range(2):  # Newton: y *= 1.5 - 0.5*ss*y^2
                    nc.vector.tensor_mul(t1h, yh, yh)
                    nc.vector.tensor_mul(t2h, t1h, ssh)
                    nc.vector.tensor_scalar(t1h, t2h, -0.5, 1.5, mult, addop)
                    nc.vector.tensor_mul(yh, yh, t1h)
                nc.vector.tensor_scalar_mul(out_scale[:, lo:hi], yh, S)

            def _ph1_chunk(lo, hi):
                nc.vector.tensor_mul(
                    sq_scr[:, : hi - lo, :],
                    xf_sb[:, lo:hi, :],
                    xf_sb[:, lo:hi, :],
                )
                nc.vector.reduce_sum(
                    ss[:, lo:hi], sq_scr[:, : hi - lo, :], axis=mybir.AxisListType.X
                )
                _rsqrt(lo, hi, sr)

            # ---- phase 2: logits + exp; row-sums split ACT-accum / DVE ----
            esum = stat.tile([128, NT * NG], _F32)

            def _exp_iter(gi, w, t, force_accum=False):
                c0 = sum(CG[:gi])
                it = gi * NT + t
                pt = psum.tile([128, 2048], _F32, tag="pt")
                for b0 in range(0, w, 512):
                    bw = min(512, w - b0)
                    for k in range(KT):
                        nc.tensor.matmul(
                            pt[:, b0 : b0 + bw],
                            xt_sb[:, k, t * 128 : (t + 1) * 128],
                            wt_sb[:, k, c0 + b0 : c0 + b0 + bw],
                            start=(k == 0),
                            stop=(k == KT - 1),
                        )
                idx = t * NG + gi
                eo = expb.tile([128, 2048], _BF16, tag="ex")
                if force_accum or it % ACT_ACCUM_EVERY == 0:
                    act = nc.scalar.activation(
                        eo[:, :w],
                        pt[:, :w],
                        AF.Exp,
                        scale=sr[:, t : t + 1],
                        accum_out=esum[:, idx : idx + 1],
                    )
                else:
                    act = nc.scalar.activation(
                        eo[:, :w], pt[:, :w], AF.Exp, scale=sr[:, t : t + 1]
                    )
                    nc.vector.reduce_sum(
                        esum[:, idx : idx + 1],
                        eo[:, :w],
                        axis=mybir.AxisListType.X,
                    )
                return act

            # program order = schedule order for Tile's semaphore thresholds:
            # keep the DVE work needed by later ACTs AHEAD of those ACTs, and
            # give the first half-pass ACT-accum reductions so the DVE queue
            # stays clear for phase-1 half 1 and the target-dot chain.
            nq = max(NT // 4, 1)
            _ph1_chunk(0, nq)
            acts0 = [_exp_iter(0, CG[0], t, force_accum=True) for t in range(nq)]
            if nq < NH:
                _ph1_chunk(nq, NH)
                acts0 += [
                    _exp_iter(0, CG[0], t, force_accum=True)
                    for t in range(nq, NT // 2)
                ]

            # The scheduler's cost model doesn't see real DMA latency, so it
            # would pack the remaining stats work ahead of the first exp
            # activations, inflating the semaphore thresholds they wait on.
            # Pin the chain heads behind early stream ACTs instead.
            h1_mul = nc.vector.tensor_mul(
                sq_scr, xf_sb[:, NH:, :], xf_sb[:, NH:, :]
            )
            tile.add_dep_helper(
                h1_mul.ins, acts0[min(1, len(acts0) - 1)].ins, sync=False,
                reason="phase1-h1 after early exp stream",
            )
            nc.vector.reduce_sum(ss[:, NH:], sq_scr, axis=mybir.AxisListType.X)
            _rsqrt(NH, NT, sr)

            rawt = stat.tile([128, NT], _F32)
            for h in range(2):
                lo, hi = h * NH, (h + 1) * NH
                rmul = nc.vector.tensor_mul(
                    sq_scr, xf_sb[:, lo:hi, :], g_sb[:, lo:hi, :]
                )
                tile.add_dep_helper(
                    rmul.ins, acts0[min(3 + h, len(acts0) - 1)].ins, sync=False,
                    reason="target-dot after early exp stream",
                )
                nc.vector.reduce_sum(
                    rawt[:, lo:hi], sq_scr, axis=mybir.AxisListType.X
                )
            st = stat.tile([128, NT], _F32)
            nc.vector.tensor_mul(st, sr, rawt)             # S * tgt
            num = stat.tile([128, NT], _F32)
            nc.vector.tensor_scalar_add(num, st, -S * M)   # S * (tgt - M)

            for t in range(NT // 2, NT):
                _exp_iter(0, CG[0], t)
            rest = [(gi, CG[gi]) for gi in range(1, NG)]
            if rest:
                gi1, w1 = rest[0]
                for t in range(NT):
                    _exp_iter(gi1, w1, t)

            expn = stat.tile([128, NT], _F32)
            nc.scalar.activation(expn, num, AF.Exp)
            expt = stat.tile([128, NT], _F32)
            nc.scalar.activation(expt, st, AF.Exp)

            # ---- phase 3 (split in t-halves so half 0 runs under the last
            # stream iterations): combine, log via DVE bit-trick (no ACT
            # table reload on the tail), write out ----
            esum_v = esum.rearrange("p (t g) -> p t g", g=NG)
            et = stat.tile([128, NT], _F32)
            denom = stat.tile([128, NT], _F32)
            ef = stat.tile([128, NT], _F32)
            mm = stat.tile([128, NT], _F32)
            acc = stat.tile([128, NT], _F32)
            L = stat.tile([128, NT], _F32)
            lsr = mybir.AluOpType.logical_shift_right
            band = mybir.AluOpType.bitwise_and
            bor = mybir.AluOpType.bitwise_or
            # log2(m) via degree-4 poly (max abs err 3.4e-4)
            PC = [
                -7.6181190215e-02, 6.0924791153e-01, -2.0338020960e+00,
                3.9788399321e+00, -2.4777674281e+00,
            ]

            def _phase3_half(h):
                lo, hi = h * NH, (h + 1) * NH
                sl = slice(lo, hi)
                nc.vector.reduce_sum(
                    et[:, sl], esum_v[:, sl, :], axis=mybir.AxisListType.X
                )
                nc.vector.tensor_add(denom[:, sl], et[:, sl], expn[:, sl])
                nc.vector.tensor_sub(denom[:, sl], denom[:, sl], expt[:, sl])
                # ln(d) = ln2 * (e + log2(m)), d = m * 2^e, m in [1,2)
                nc.vector.tensor_scalar(
                    acc[:, sl].bitcast(_I32), denom[:, sl].bitcast(_I32),
                    23, None, lsr,
                )
                nc.vector.tensor_scalar(
                    acc[:, sl].bitcast(_I32), acc[:, sl].bitcast(_I32),
                    127, None, sub,
                )
                nc.vector.tensor_copy(ef[:, sl], acc[:, sl].bitcast(_I32))
                nc.vector.tensor_scalar(
                    mm[:, sl].bitcast(_I32), denom[:, sl].bitcast(_I32),
                    0x7FFFFF, 0x3F800000, band, bor,
                )
                nc.vector.tensor_scalar(
                    acc[:, sl], mm[:, sl], PC[0], PC[1], mult, addop
                )
                for ci in PC[2:]:
                    nc.vector.tensor_mul(acc[:, sl], acc[:, sl], mm[:, sl])
                    nc.vector.tensor_scalar_add(acc[:, sl], acc[:, sl], ci)
                nc.vector.tensor_add(acc[:, sl], acc[:, sl], ef[:, sl])
                nc.vector.tensor_scalar_mul(acc[:, sl], acc[:, sl], float(np.log(2.0)))
                nc.vector.tensor_sub(L[:, sl], num[:, sl], acc[:, sl])

            if rest[1:]:
                for gi, w in rest[1:-1]:
                    for t in range(NT):
                        _exp_iter(gi, w, t)
                gi, w = rest[-1]
                for t in range(NT // 2):
                    _exp_iter(gi, w, t)
                _phase3_half(0)
                for t in range(NT // 2, NT):
                    _exp_iter(gi, w, t)
                _phase3_half(1)
            else:
                _phase3_half(0)
                _phase3_half(1)
            nc.sync.dma_start(out=out_ext[:], in_=L)

    nc.finalize()
    return nc


_NC_CACHE = None


def _get_nc():
    global _NC_CACHE
    if _NC_CACHE is None:
        _NC_CACHE = _build_nc()
    return _NC_CACHE


def _shuffle_pm(a, nt):
    """[nt*128, d] row-major -> [128, nt, d] partition-major."""
    d = a.shape[-1]
    return np.ascontiguousarray(a.reshape(nt, 128, d).transpose(1, 0, 2))


def prep_core(xs, ls, W, wt=None):
    """Build one core's input map from its row block. Layouts partition-major."""
    nt = xs.shape[0] // 128
    c = W.shape[0]
    if wt is None:
        wt = _shuffle_pm(np.ascontiguousarray(W.T), KT).astype(ml_dtypes.bfloat16)
    xt = _shuffle_pm(np.ascontiguousarray(xs.T), KT).astype(ml_dtypes.bfloat16)
    xf = _shuffle_pm(xs, nt).astype(ml_dtypes.bfloat16)
    g = _shuffle_pm(W[ls], nt).astype(ml_dtypes.bfloat16)
    return {"xt": xt, "wt": wt, "xf": xf, "g": g}


def make_in_maps(x, labels, W):
    x = np.asarray(x, dtype=np.float32)
    W = np.asarray(W, dtype=np.float32)
    labels = np.asarray(labels)
    wt = _shuffle_pm(np.ascontiguousarray(W.T), KT).astype(ml_dtypes.bfloat16)
    return [
        prep_core(
            x[i * NS : (i + 1) * NS], labels[i * NS : (i + 1) * NS], W, wt
        )
        for i in range(NCORES)
    ]


def run_device(x, labels, W, **kwargs):
    nc = _get_nc()
    in_maps = make_in_maps(x, labels, W)
    res = run_bass_kernel_spmd(nc, in_maps, list(range(NCORES)), **kwargs)
    return res


def finish(res):
    parts = []
    for i in range(NCORES):
        o = res.results[i]["out"]            # [128, NT]; row = t*128 + p
        parts.append(np.asarray(o).T.reshape(-1))
    L = np.concatenate(parts)
    return np.asarray(-np.mean(L), dtype=np.float32)


def kernel(x, labels, W):
    return finish(run_device(x, labels, W))
